# revision 1
# baseline (speedup 1.0000x reference)
"""Multi-head self-attention (RoPE, causal) Trainium2 Bass kernel.

Problem: B=4, S=2048, D=1024, H=16 heads, hd=64, fused QKV + RoPE +
causal softmax attention + output projection (torch-Linear convention).

Sharding: Megatron-style tensor parallel over heads. Each of the 8
NeuronCores owns 2 heads: it projects the full token stream through its
128-row slices of Wq/Wk/Wv, applies RoPE, runs causal attention for its
2 heads x 4 batches, and computes a partial output projection
h_core @ Wo[:, core_slice].T  (transposed layout). The host sums the 8
partial outputs and adds the output bias.

v2 design notes (vs the first working version):
  - P@V runs V-stationary: out_hT[d, q] = sum_k V[k, d] * P_T[k, q],
    streaming wide q chunks (512) with partial-width matmuls on the
    triangular boundary, so the PE streams exactly the causal triangle
    instead of being LDWEIGHTS-bound on P-stationary 65-col matmuls.
    The softmax denominator rides along as a ones column in V (psum
    row 64); normalization = PE row-broadcast of the den row (ones[1,64]
    stationary) + DVE reciprocal + multiply.  hT is produced directly,
    removing the hqb transposes from the scalar queue.
  - V is produced transposed (wv-stationary, stream-bound) and moved to
    natural [tok, d] layout with DMA transposes on the *sync* queue so
    the scalar queue stays dedicated to the critical-path exp.
  - Output DMAs are issued from the gpsimd queue.
  - All phases are emitted through a fine-grained interleaver (score
    groups of problem i | P@V chunks of problem i-1 | projection /
    O-proj filler pieces) so the PE never idles long enough for the
    HAM clock gate to re-throttle it to 1.2 GHz.
"""

import os
import sys

for _p in ("/opt/trn_rl_repo",):
    if os.path.isdir(_p) and _p not in sys.path:
        sys.path.append(_p)

import math

import ml_dtypes
import numpy as np

import concourse.bass as bass
import concourse.mybir as mybir
import concourse.tile as tile
from concourse import bacc
from concourse.bass import ts, ds
from concourse.bass_utils import run_bass_kernel_spmd

BF16 = ml_dtypes.bfloat16

B = 4
S = 2048
D = 1024
H = 16
HD = 64
NCORES = 8
HPC = H // NCORES          # heads per core = 2
PC = HPC * HD              # partition rows per core's heads = 128
T = B * S                  # 8192 tokens
KT = D // 128              # f_in k-tiles = 8
NTOK = T // 128            # 64 token tiles of 128
SCALE = 1.0 / math.sqrt(HD)
ROPE_THETA = 10000.0

TWO_PI = 2.0 * math.pi
INV_2PI = 1.0 / TWO_PI
MAGIC = 12582912.0         # 1.5 * 2**23, float32 round-to-nearest trick
HALF_PI = math.pi / 2.0

NQT = S // 128             # 16 q/k tiles per sequence
# triangular packing offsets for P_T: row kt covers q in [kt*128, S)
OFFS = [0] * NQT
for _kt in range(1, NQT):
    OFFS[_kt] = OFFS[_kt - 1] + (S - (_kt - 1) * 128)
PTRI_W = OFFS[-1] + (S - (NQT - 1) * 128)   # 17408

TC = 512                   # token chunk for projections
NTC = T // TC              # 16
CPB = S // TC              # proj chunks per batch = 4
QC = 512                   # P@V q-chunk width
NPV = S // QC              # P@V chunks per problem = 4
SGW = 512                  # scores psum group width


def _build_nc():
    nc = bacc.Bacc("TRN2", target_bir_lowering=False, debug=False,
                   num_devices=NCORES)
    dt = mybir.dt

    # ---- I/O ----
    x_in = nc.dram_tensor("x", [NTC, 128, KT * TC], dt.bfloat16,
                          kind="ExternalInput")
    pos_in = nc.dram_tensor("pos", [S], dt.int32, kind="ExternalInput")
    wq_in = nc.dram_tensor("wq", [D, PC], dt.bfloat16, kind="ExternalInput")
    wk_in = nc.dram_tensor("wk", [D, PC], dt.bfloat16, kind="ExternalInput")
    wv_in = nc.dram_tensor("wv", [D, PC], dt.bfloat16, kind="ExternalInput")
    wo_in = nc.dram_tensor("wo", [PC, D], dt.bfloat16, kind="ExternalInput")
    bq_in = nc.dram_tensor("bq", [PC], dt.float32, kind="ExternalInput")
    bk_in = nc.dram_tensor("bk", [PC], dt.float32, kind="ExternalInput")
    bv_in = nc.dram_tensor("bv", [PC], dt.float32, kind="ExternalInput")
    out_d = nc.dram_tensor("out", [KT, T // 512, 128, 512], dt.bfloat16,
                           kind="ExternalOutput")
    KDEBUG = os.environ.get("KDEBUG") == "1"
    if KDEBUG:
        dbg_q = nc.dram_tensor("dbg_q", [128, T], dt.bfloat16, kind="ExternalOutput")
        dbg_k = nc.dram_tensor("dbg_k", [128, T], dt.bfloat16, kind="ExternalOutput")
        dbg_h = nc.dram_tensor("dbg_h", [128, T], dt.bfloat16, kind="ExternalOutput")
        dbg_v = nc.dram_tensor("dbg_v", [128, NTOK * 160], dt.bfloat16,
                               kind="ExternalOutput")
        dbg_pt = nc.dram_tensor("dbg_pt", [128, PTRI_W], dt.bfloat16,
                                kind="ExternalOutput")
        dbg_mb = nc.dram_tensor("dbg_mb", [128, NQT * 128], dt.bfloat16,
                                kind="ExternalOutput")


    # ---- inline constants ----
    # RT = R.T where (R @ q)[2i] = -q[2i+1], (R @ q)[2i+1] = q[2i],
    # block-diagonal over the 2 stacked heads (structure identical).
    r = np.zeros((PC, PC), dtype=np.float32)
    for h in range(HPC):
        for i in range(HD // 2):
            r[h * HD + 2 * i, h * HD + 2 * i + 1] = -1.0
            r[h * HD + 2 * i + 1, h * HD + 2 * i] = 1.0
    rt_np = np.ascontiguousarray(r.T).astype(BF16)
    # causal mask for diagonal scoresT blocks: keep k_local <= q_local
    mask_np = np.tril(np.ones((128, 128), dtype=np.float32)).T.astype(BF16)
    # inv_freq per partition: p -> head-local pair (p % 64) // 2
    invf_np = np.zeros((PC, 1), dtype=np.float32)
    for p in range(PC):
        i = (p % HD) // 2
        invf_np[p, 0] = 1.0 / (ROPE_THETA ** (2.0 * i / HD))

    rt_d = nc.inline_tensor(rt_np, "rt_c")
    # row-selector x ones for the rec broadcast: ejs[p, qt*64+m] = (p == qt)
    ejs_np = np.zeros((16, 16 * HD), dtype=np.float16)
    for j in range(16):
        ejs_np[j, j * HD:(j + 1) * HD] = 1.0
    ejs_d = nc.inline_tensor(ejs_np, "ejs_c")
    idf_np = np.eye(128, dtype=np.float32)
    idf_d = nc.inline_tensor(idf_np, "idf_c")
    mask_d = nc.inline_tensor(mask_np, "mask_c")
    invf_d = nc.inline_tensor(invf_np, "invf_c")

    fp32 = dt.float32
    bf16 = dt.bfloat16

    with tile.TileContext(nc) as tc:
        with (
            tc.tile_pool(name="consts", bufs=1) as consts,
            tc.tile_pool(name="resid", bufs=1) as resid,
            tc.tile_pool(name="xp", bufs=2) as xp,
            tc.tile_pool(name="work", bufs=2) as work,
            tc.tile_pool(name="vst", bufs=2) as vst,
            tc.tile_pool(name="stg", bufs=2) as stg,
            tc.tile_pool(name="htu", bufs=5) as htu,
            tc.tile_pool(name="csw", bufs=1) as csw,
            tc.tile_pool(name="ptri", bufs=2) as ptri_pool,
            tc.tile_pool(name="bigps", bufs=2, space="PSUM") as bigps,
            tc.tile_pool(name="pvps", bufs=2, space="PSUM") as pvps,
            tc.tile_pool(name="recps", bufs=1, space="PSUM") as recps,
            tc.tile_pool(name="r16ps", bufs=1, space="PSUM") as r16ps,
            tc.tile_pool(name="accps", bufs=2, space="PSUM") as accps,
        ):
            # ---- load constants / weights to SBUF ----
            # x chunk 0 DMA first so the PE can start ASAP.
            xt0 = xp.tile([128, KT, TC], bf16, tag="xt")
            nc.scalar.dma_start(out=xt0.rearrange("p a b -> p (a b)"),
                                in_=x_in[0, :, :])

            wq_sb = consts.tile([128, KT, PC], bf16, tag="wq")
            wk_sb = consts.tile([128, KT, PC], bf16, tag="wk")
            wv_sb = consts.tile([128, KT, PC], bf16, tag="wv")
            for t_sb, t_d in ((wq_sb, wq_in), (wk_sb, wk_in), (wv_sb, wv_in)):
                nc.sync.dma_start(
                    out=t_sb, in_=t_d.ap().rearrange("(kt p) m -> p kt m", p=128))
            wo_sb = consts.tile([128, D], bf16, tag="wo")
            nc.sync.dma_start(out=wo_sb, in_=wo_in[:, :])
            rt_sb = consts.tile([128, 128], bf16, tag="rt")
            nc.sync.dma_start(out=rt_sb, in_=rt_d[:, :])
            mask_sb = consts.tile([128, 128], bf16, tag="mask")
            nc.sync.dma_start(out=mask_sb, in_=mask_d[:, :])
            ejs_sb = consts.tile([16, 16 * HD], dt.float16, tag="ejs")
            nc.sync.dma_start(out=ejs_sb, in_=ejs_d[:, :])
            idf_sb = consts.tile([128, 128], fp32, tag="idf")
            nc.sync.dma_start(out=idf_sb, in_=idf_d[:, :])
            invf_sb = consts.tile([128, 1], fp32, tag="invf")
            nc.sync.dma_start(out=invf_sb, in_=invf_d[:, :])
            bq_sb = consts.tile([128, 1], fp32, tag="bq")
            nc.sync.dma_start(out=bq_sb, in_=bq_in.ap().rearrange("(p o) -> p o", o=1))
            bk_sb = consts.tile([128, 1], fp32, tag="bk")
            nc.sync.dma_start(out=bk_sb, in_=bk_in.ap().rearrange("(p o) -> p o", o=1))
            bv_sb = consts.tile([128, 1], fp32, tag="bv")
            nc.sync.dma_start(out=bv_sb, in_=bv_in.ap().rearrange("(p o) -> p o", o=1))
            halfpi_sb = consts.tile([128, 1], fp32, tag="halfpi")
            nc.vector.memset(halfpi_sb, HALF_PI)


            # ---- residents (per batch, so range tracking cannot create
            # false cross-batch dependencies) ----
            qT, kT, hT = [], [], []
            for bb in range(B):
                tq = resid.tile([128, S], bf16, tag=f"qT{bb}")
                qT.append(tq)
                tk = resid.tile([128, S], bf16, tag=f"kT{bb}")
                kT.append(tk)
                th = resid.tile([128, S], bf16, tag=f"hT{bb}")
                hT.append(th)
            # v natural as repeating [ones(64) | d_h0(64) | d_h1(64)]
            # 192-col blocks (plus one trailing ones block): head0's P@V
            # lhsT is [ones|d0] (den in psum rows 0-63, h in 64-127) and
            # head1's is [d1|ones-of-next-block] (h in 0-63, den 64-127) --
            # both plain contiguous 128-col slices.
            NTB = NTOK // B            # 16 tok tiles per batch
            VW = NTB * 192 + 64
            vN = []
            for bb in range(B):
                tv = resid.tile([128, VW], bf16, tag=f"vN{bb}")
                vN.append(tv)
                nc.vector.memset(
                    bass.AP(tensor=tv.tensor, offset=tv.offset,
                            ap=[tv.ap[0], [192, NTB + 1], [1, 64]]), 1.0)
            # RoPE cos/sin tables [128, S] bf16, built from positions
            cos_sb = consts.tile([128, S], bf16, tag="cosT")
            sin_sb = consts.tile([128, S], bf16, tag="sinT")

            CS_CH = 256

            def cs_piece(ci):
                """Build cos/sin for position chunk ci (range-reduced Sin)."""
                sl = ts(ci, CS_CH)
                posi = csw.tile([128, CS_CH], dt.int32, tag="posi")
                nc.sync.dma_start(
                    out=posi,
                    in_=bass.AP(tensor=pos_in, offset=ci * CS_CH,
                                ap=[[0, 128], [1, CS_CH]]))
                posf = csw.tile([128, CS_CH], fp32, tag="posf")
                nc.vector.tensor_copy(posf, posi)
                ang = csw.tile([128, CS_CH], fp32, tag="ang")
                nc.vector.tensor_scalar_mul(ang, posf, invf_sb)
                # sin: reduce ang to [-pi, pi]
                rnd = csw.tile([128, CS_CH], fp32, tag="rnd")
                red = csw.tile([128, CS_CH], fp32, tag="red")
                nc.vector.tensor_scalar(rnd, ang, INV_2PI, MAGIC,
                                        mybir.AluOpType.mult,
                                        mybir.AluOpType.add)
                nc.vector.tensor_scalar(rnd, rnd, MAGIC, None,
                                        mybir.AluOpType.subtract)
                nc.vector.scalar_tensor_tensor(
                    red, rnd, -TWO_PI, ang,
                    op0=mybir.AluOpType.mult, op1=mybir.AluOpType.add)
                nc.scalar.activation(sin_sb[:, sl], red,
                                     mybir.ActivationFunctionType.Sin)
                # cos(x) = sin(y + pi/2), y = x - 2pi*round((x+pi/2)/2pi)
                nc.vector.tensor_scalar(rnd, ang, INV_2PI, MAGIC + 0.25,
                                        mybir.AluOpType.mult,
                                        mybir.AluOpType.add)
                nc.vector.tensor_scalar(rnd, rnd, MAGIC, None,
                                        mybir.AluOpType.subtract)
                nc.vector.scalar_tensor_tensor(
                    red, rnd, -TWO_PI, ang,
                    op0=mybir.AluOpType.mult, op1=mybir.AluOpType.add)
                nc.scalar.activation(cos_sb[:, sl], red,
                                     mybir.ActivationFunctionType.Sin,
                                     bias=halfpi_sb[:, :])

            # ---- phase 1: QKV projection + RoPE, per token chunk ----
            xt_cache = {0: xt0}

            def get_xt(tci):
                if tci not in xt_cache:
                    xt = xp.tile([128, KT, TC], bf16, tag="xt")
                    nc.scalar.dma_start(out=xt.rearrange("p a b -> p (a b)"),
                                        in_=x_in[tci, :, :])
                    xt_cache[tci] = xt
                return xt_cache[tci]

            def proj_qk_mm(tci, which, st):
                """q/k projection matmuls for one 512-token chunk."""
                xt = get_xt(tci)
                if tci + 1 < NTC:
                    get_xt(tci + 1)
                w_sb, b_sb = ((wq_sb, bq_sb) if which == "q"
                              else (wk_sb, bk_sb))
                pa = accps.tile([128, TC], fp32, tag="acc")
                for kt in range(KT):
                    nc.tensor.matmul(pa, lhsT=w_sb[:, kt, :],
                                     rhs=xt[:, kt, :],
                                     start=(kt == 0), stop=(kt == KT - 1))
                a_sb = work.tile([128, TC], bf16, tag="a_sb")
                nc.vector.tensor_scalar_add(a_sb, pa, b_sb)
                st[which] = a_sb

            def proj_qk_rope(tci, which, st):
                """RoPE for the chunk: rotation matmul + combine."""
                dl = qT if which == "q" else kT
                dest = dl[tci // CPB]
                a_sb = st.pop(which)
                pb = accps.tile([128, TC], fp32, tag="acc")
                nc.tensor.matmul(pb, lhsT=rt_sb, rhs=a_sb,
                                 start=True, stop=True)
                ssl = ds((tci * TC) % S, TC)
                t1 = work.tile([128, TC], bf16, tag="t1")
                nc.vector.tensor_mul(t1, a_sb, cos_sb[:, ssl])
                t2 = work.tile([128, TC], bf16, tag="t2")
                nc.vector.tensor_mul(t2, pb, sin_sb[:, ssl])
                nc.vector.tensor_add(dest[:, ts(tci % CPB, TC)], t1, t2)

            def proj_v(tci):
                """v projection, transposed production (wv stationary),
                then DMA-transposed into natural layout vA on sync."""
                xt = get_xt(tci)
                pv = accps.tile([128, TC], fp32, tag="acc")
                for kt in range(KT):
                    nc.tensor.matmul(pv, lhsT=wv_sb[:, kt, :],
                                     rhs=xt[:, kt, :],
                                     start=(kt == 0), stop=(kt == KT - 1))
                vTst = vst.tile([128, TC], bf16, tag="vTst")
                nc.vector.tensor_scalar_add(vTst, pv, bv_sb)
                for sub in range(TC // 128):
                    tl = (tci % CPB) * (TC // 128) + sub
                    nc.sync.dma_start_transpose(
                        vN[tci // CPB][:, ds(192 * tl + 64, 128)],
                        vTst[:, ts(sub, 128)])

            def proj_pieces(b):
                out = []
                for cc in range(CPB):
                    tci = b * CPB + cc
                    st = {}
                    out.append(lambda t=tci, s=st: proj_qk_mm(t, "q", s))
                    out.append(lambda t=tci, s=st: proj_qk_rope(t, "q", s))
                    out.append(lambda t=tci, s=st: proj_qk_mm(t, "k", s))
                    out.append(lambda t=tci, s=st: proj_qk_rope(t, "k", s))
                    out.append(lambda t=tci: proj_v(t))
                return out

            # ---- phase 2a: scores + exp + mask for one (batch, head) ----
            def score_pieces(i, pt):
                """Closures, each = one <=512-col psum group: matmul the
                row pieces intersecting the window, exp into pt, mask
                any diagonal blocks fully inside the window."""
                b, h = i // HPC, i % HPC
                hsl = ds(h * HD, HD)
                pieces = []
                x0 = 0
                while x0 < PTRI_W:
                    w = min(SGW, PTRI_W - x0)

                    def piece(x0=x0, w=w):
                        sc = bigps.tile([128, SGW], fp32, tag="big")
                        # rows intersecting flat-span window [x0, x0+w)
                        for kt in range(NQT):
                            r0, r1 = OFFS[kt], OFFS[kt] + (S - kt * 128)
                            lo, hi = max(r0, x0), min(r1, x0 + w)
                            if lo >= hi:
                                continue
                            q0 = kt * 128 + (lo - r0)
                            # split at 512-aligned psum columns: a matmul
                            # output must not cross a PSUM bank boundary
                            cuts = [lo]
                            nb = (lo - x0) // 512 * 512 + 512 + x0
                            while nb < hi:
                                cuts.append(nb)
                                nb += 512
                            cuts.append(hi)
                            for aa, bb in zip(cuts, cuts[1:]):
                                nc.tensor.matmul(
                                    sc[:, ds(aa - x0, bb - aa)],
                                    lhsT=kT[b][hsl, ds(kt * 128, 128)],
                                    rhs=qT[b][hsl, ds(q0 + (aa - lo), bb - aa)],
                                    start=True, stop=True)
                        nc.scalar.activation(
                            pt[:, ds(x0, w)], sc[:, 0:w],
                            mybir.ActivationFunctionType.Exp, scale=SCALE)
                        for kt in range(NQT):
                            if x0 <= OFFS[kt] and OFFS[kt] + 128 <= x0 + w:
                                dsl = ds(OFFS[kt], 128)
                                nc.gpsimd.tensor_mul(
                                    pt[:, dsl], pt[:, dsl], mask_sb)

                    pieces.append(piece)
                    x0 += w
                return pieces

            # ---- phase 2b: P@V (V stationary) + normalize into hT ----
            # Normalization: den rows (psum row 64, the ones-column sums)
            # are collected into a [16,128] f16 tile, DMA-transposed into
            # q-partition layout where the DVE reciprocal is cheap
            # ([128,16] = 16 elems/lane), transposed back on the PE, and
            # row-broadcast via selector matmuls; h_u is staged to SBUF so
            # the final multiply has a single PSUM operand (recB).
            def pv_pieces(i, pt):
                b, h = i // HPC, i % HPC
                st = {}
                pieces = []

                def mm_piece(c):
                    q0 = c * QC
                    acc = pvps.tile([128, QC], fp32, tag="pv")
                    kts = [kt for kt in range(NQT) if kt * 128 < q0 + QC]
                    for j, kt in enumerate(kts):
                        lo = max(kt * 128, q0)
                        w = q0 + QC - lo
                        lt = vN[b][:, ds(192 * kt + 128 * h, 128)]
                        nc.tensor.matmul(
                            acc[:, ds(lo - q0, w)], lhsT=lt,
                            rhs=pt[:, ds(OFFS[kt] + lo - kt * 128, w)],
                            start=(j == 0), stop=(j == len(kts) - 1))
                    if c == 0:
                        d16 = stg.tile([16, 128], dt.float16, tag="den16")
                        st["den16"] = d16
                    hu = htu.tile([HD, QC], bf16, tag="hTu")
                    nc.vector.tensor_copy(hu, acc[ds(64 - 64 * h, HD), :])
                    st[("hu", c)] = hu
                    dnr = stg.tile([1, QC], dt.float16, tag="denrow")
                    nc.vector.tensor_copy(dnr, acc[ds(64 * h, 1), :])
                    nc.sync.dma_start(
                        out=st["den16"][ds(4 * c, 4), :], in_=dnr)

                def rec_piece():
                    denT = stg.tile([128, 16], dt.float16, tag="denT")
                    nc.sync.dma_start_transpose(denT, st["den16"])
                    recT = stg.tile([128, 16], fp32, tag="recT")
                    nc.vector.reciprocal(recT, denT)
                    r16 = r16ps.tile([16, 128], fp32, tag="r16")
                    nc.tensor.transpose(r16, recT, idf_sb)
                    r16s = stg.tile([16, 128], dt.float16, tag="r16s")
                    nc.vector.tensor_copy(r16s, r16)
                    for c in range(NPV):
                        recb = recps.tile([HD, QC], fp32, tag="recb")
                        for j in range(4):
                            qt = 4 * c + j
                            nc.tensor.matmul(
                                recb[:, ts(j, 128)],
                                lhsT=ejs_sb[:, ds(qt * HD, HD)],
                                rhs=r16s, start=True, stop=True)
                        nc.vector.tensor_mul(
                            hT[b][ds(h * HD, HD), ds(c * QC, QC)],
                            st[("hu", c)], recb)

                for c in range(NPV):
                    pieces.append(lambda c=c: mm_piece(c))
                pieces.append(rec_piece)
                return pieces

            # ---- phase 3: output projection (partial, transposed) ----
            def oproj_piece(b, ft):
                base = b * S
                for cc in range(S // 512):
                    po = accps.tile([128, 512], fp32, tag="acc")
                    nc.tensor.matmul(
                        po, lhsT=wo_sb[:, ts(ft, 128)],
                        rhs=hT[b][:, ds(cc * 512, 512)],
                        start=True, stop=True)
                    ostage = work.tile([128, 512], bf16, tag="ostage")
                    if (ft + cc) % 4 == 1:
                        nc.scalar.copy(ostage, po)
                    else:
                        nc.vector.tensor_copy(ostage, po)
                    nc.gpsimd.dma_start(
                        out=out_d[ft, b * (S // 512) + cc, :, :], in_=ostage)

            def oproj_pieces(b, fts):
                return [lambda f=ft, bb=b: oproj_piece(bb, f) for ft in fts]

            # ---- emission schedule ----
            def interleave(main, others, ratio=None):
                """Emit main pieces with `others` spliced evenly."""
                if ratio is None:
                    ratio = max(1, len(main) // max(1, len(others)))
                oi = 0
                for n, m in enumerate(main):
                    m()
                    if n % ratio == ratio - 1 and oi < len(others):
                        others[oi]()
                        oi += 1
                for o in others[oi:]:
                    o()

            # startup: batch-0 projection with cos/sin builds interleaved
            p0 = proj_pieces(0)
            for n, p in enumerate(p0):
                if n % 5 == 0 and n // 5 < 4:
                    cs_piece(2 * (n // 5))
                    cs_piece(2 * (n // 5) + 1)
                p()

            nprob = B * HPC
            pts = {}
            pvq = []      # pending P@V pieces (from previous problem)
            for i in range(nprob + 1):
                fillers = list(pvq)
                pvq = []
                # projection for batch i//2+1 split over iterations 2b-2,
                # 2b-1; O-proj for batch (i-3)//2 split over 2b+3, 2b+4
                bb = i // 2 + 1
                half = i % 2
                if bb < B:
                    fillers += proj_pieces(bb)[10 * half:10 * half + 10]
                if i >= 3:
                    ob = (i - 3) // 2
                    ohalf = (i - 3) % 2
                    fillers += oproj_pieces(ob, range(4 * ohalf, 4 * ohalf + 4))
                if i == nprob:   # epilogue: last batch's O-proj
                    fillers += oproj_pieces(B - 1, range(8))
                if i < nprob:
                    pt = ptri_pool.tile([128, PTRI_W], bf16, tag="pt")
                    pts[i] = pt
                    interleave(score_pieces(i, pt), fillers)
                    pvq = pv_pieces(i, pt)
                else:
                    interleave(fillers, [])
            # pv of the last problem ran inside the epilogue fillers
            if KDEBUG:
                for bb in range(B):
                    nc.sync.dma_start(out=dbg_q[:, ts(bb, S)], in_=qT[bb])
                    nc.sync.dma_start(out=dbg_k[:, ts(bb, S)], in_=kT[bb])
                    nc.sync.dma_start(out=dbg_h[:, ts(bb, S)], in_=hT[bb])

    nc.compile()
    return nc


_NC_CACHE = None


def _get_nc():
    global _NC_CACHE
    if _NC_CACHE is None:
        _NC_CACHE = _build_nc()
    return _NC_CACHE


def build_in_maps(x, positions, Wqkv, bqkv, Wo, bo):
    xT = x.reshape(T, D).T.astype(BF16)            # [D, T]
    # chunk-block layout [tci, p, kt*512]: contiguous 4KB runs per partition
    xblk = np.ascontiguousarray(
        xT.reshape(KT, 128, NTC, TC).transpose(2, 1, 0, 3).reshape(NTC, 128, KT * TC))
    pos = np.ascontiguousarray(positions[0]).astype(np.int32)
    in_maps = []
    for c in range(NCORES):
        r0 = c * PC
        wq = np.ascontiguousarray(Wqkv[r0:r0 + PC, :].T).astype(BF16)
        wk = np.ascontiguousarray(Wqkv[D + r0:D + r0 + PC, :].T).astype(BF16)
        wv = np.ascontiguousarray(Wqkv[2 * D + r0:2 * D + r0 + PC, :].T).astype(BF16)
        wo = np.ascontiguousarray(Wo[:, r0:r0 + PC].T).astype(BF16)
        in_maps.append({
            "x": xblk, "pos": pos,
            "wq": wq, "wk": wk, "wv": wv, "wo": wo,
            "bq": bqkv[r0:r0 + PC].astype(np.float32),
            "bk": bqkv[D + r0:D + r0 + PC].astype(np.float32),
            "bv": bqkv[2 * D + r0:2 * D + r0 + PC].astype(np.float32),
        })
    return in_maps


def assemble_out(res, bo):
    acc = res.results[0]["out"].astype(np.float32)
    for c in range(1, NCORES):
        acc += res.results[c]["out"].astype(np.float32)
    # [KT, T//512, 128, 512] -> [D, T]
    full = acc.transpose(0, 2, 1, 3).reshape(D, T)
    out = full + bo[:, None].astype(np.float32)
    return np.ascontiguousarray(out.T).reshape(B, S, D)


def kernel(x, positions, Wqkv, bqkv, Wo, bo):
    x = np.asarray(x)
    positions = np.asarray(positions)
    Wqkv = np.asarray(Wqkv)
    bqkv = np.asarray(bqkv)
    Wo = np.asarray(Wo)
    bo = np.asarray(bo)
    nc = _get_nc()
    in_maps = build_in_maps(x, positions, Wqkv, bqkv, Wo, bo)
    res = run_bass_kernel_spmd(nc, in_maps, core_ids=list(range(NCORES)))
    return assemble_out(res, bo)



# revision 6
# speedup vs baseline: 1.0529x; 1.0529x over previous
"""Multi-head self-attention (RoPE, causal) Trainium2 Bass kernel.

Problem: B=4, S=2048, D=1024, H=16 heads, hd=64, fused QKV + RoPE +
causal softmax attention + output projection (torch-Linear convention).

Sharding: Megatron-style tensor parallel over heads. Each of the 8
NeuronCores owns 2 heads: it projects the full token stream through its
128-row slices of Wq/Wk/Wv, applies RoPE, runs causal attention for its
2 heads x 4 batches, and computes a partial output projection
h_core @ Wo[:, core_slice].T  (transposed layout). The host sums the 8
partial outputs and adds the output bias.

v3 design notes (vs v2):
  - Score psum groups widened to 1024 (2 psum banks, double buffered =
    4 banks) so each exp ACTIVATE covers 1024 columns -> half the
    per-instruction overhead on the scalar engine (136 calls, not 272).
  - The softmax denominator reciprocal broadcast is restructured: den
    rows gather to [16,128], DMA-transpose to [128,16], DVE reciprocal
    (fp16), then ONE plain DMA scatters it into a [1, S] rec_row; the
    per-chunk [64,512] broadcast is a single K=1 matmul with a ones
    vector riding the pv psum pool. This drops the ejs/idf selector
    matmuls (128 MMs), the PE transpose, and two dedicated psum banks.
  - P@V staging: one [128,512] fp16 CAST per chunk captures h values
    AND den rows together; hu / den are views into it (the separate
    dnr CAST is gone).
  - RoPE final add runs on gpsimd; o-proj psum->sbuf casts all run on
    DVE so the scalar engine only ever runs Sin (startup) and Exp (no
    activation-table thrash).
  - qT/kT residents rotate through 2 buffers instead of 4 per-batch
    tiles (SBUF headroom).
"""

import os
import sys

for _p in ("/opt/trn_rl_repo",):
    if os.path.isdir(_p) and _p not in sys.path:
        sys.path.append(_p)

import math

import ml_dtypes
import numpy as np

import concourse.bass as bass
import concourse.mybir as mybir
import concourse.tile as tile
from concourse import bacc
from concourse.bass import ts, ds
from concourse.bass_utils import run_bass_kernel_spmd

BF16 = ml_dtypes.bfloat16

B = 4
S = 2048
D = 1024
H = 16
HD = 64
NCORES = 8
HPC = H // NCORES          # heads per core = 2
PC = HPC * HD              # partition rows per core's heads = 128
T = B * S                  # 8192 tokens
KT = D // 128              # f_in k-tiles = 8
NTOK = T // 128            # 64 token tiles of 128
SCALE = 1.0 / math.sqrt(HD)
ROPE_THETA = 10000.0

TWO_PI = 2.0 * math.pi
INV_2PI = 1.0 / TWO_PI
MAGIC = 12582912.0         # 1.5 * 2**23, float32 round-to-nearest trick
HALF_PI = math.pi / 2.0

NQT = S // 128             # 16 q/k tiles per sequence
# triangular packing offsets for P_T: row kt covers q in [kt*128, S)
OFFS = [0] * NQT
for _kt in range(1, NQT):
    OFFS[_kt] = OFFS[_kt - 1] + (S - (_kt - 1) * 128)
PTRI_W = OFFS[-1] + (S - (NQT - 1) * 128)   # 17408

TC = 512                   # token chunk for projections
NTC = T // TC              # 16
CPB = S // TC              # proj chunks per batch = 4
QC = 512                   # P@V q-chunk width
NPV = S // QC              # P@V chunks per problem = 4
SGW = 1024                 # scores psum group width (2 banks)


def _build_nc():
    nc = bacc.Bacc("TRN2", target_bir_lowering=False, debug=False,
                   num_devices=NCORES)
    dt = mybir.dt

    # ---- I/O ----
    x_in = nc.dram_tensor("x", [NTC, 128, KT * TC], dt.bfloat16,
                          kind="ExternalInput")
    pos_in = nc.dram_tensor("pos", [S], dt.int32, kind="ExternalInput")
    wq_in = nc.dram_tensor("wq", [D, PC], dt.bfloat16, kind="ExternalInput")
    wk_in = nc.dram_tensor("wk", [D, PC], dt.bfloat16, kind="ExternalInput")
    wv_in = nc.dram_tensor("wv", [D, PC], dt.bfloat16, kind="ExternalInput")
    wo_in = nc.dram_tensor("wo", [PC, D], dt.bfloat16, kind="ExternalInput")
    bq_in = nc.dram_tensor("bq", [PC], dt.float32, kind="ExternalInput")
    bk_in = nc.dram_tensor("bk", [PC], dt.float32, kind="ExternalInput")
    bv_in = nc.dram_tensor("bv", [PC], dt.float32, kind="ExternalInput")
    out_d = nc.dram_tensor("out", [KT, T // 512, 128, 512], dt.bfloat16,
                           kind="ExternalOutput")
    KDEBUG = os.environ.get("KDEBUG") == "1"
    if KDEBUG:
        dbg_q = nc.dram_tensor("dbg_q", [128, T], dt.bfloat16, kind="ExternalOutput")
        dbg_k = nc.dram_tensor("dbg_k", [128, T], dt.bfloat16, kind="ExternalOutput")
        dbg_h = nc.dram_tensor("dbg_h", [128, T], dt.bfloat16, kind="ExternalOutput")
        dbg_pt = nc.dram_tensor("dbg_pt", [128, PTRI_W], dt.bfloat16,
                                kind="ExternalOutput")
        dbg_rr = nc.dram_tensor("dbg_rr", [1, S], dt.float32,
                                kind="ExternalOutput")

    # ---- inline constants ----
    # RT = R.T where (R @ q)[2i] = -q[2i+1], (R @ q)[2i+1] = q[2i],
    # block-diagonal over the 2 stacked heads (structure identical).
    r = np.zeros((PC, PC), dtype=np.float32)
    for h in range(HPC):
        for i in range(HD // 2):
            r[h * HD + 2 * i, h * HD + 2 * i + 1] = -1.0
            r[h * HD + 2 * i + 1, h * HD + 2 * i] = 1.0
    rt_np = np.ascontiguousarray(r.T).astype(BF16)
    # causal mask for diagonal scoresT blocks: keep k_local <= q_local
    mask_np = np.tril(np.ones((128, 128), dtype=np.float32)).T.astype(BF16)
    # inv_freq per partition: p -> head-local pair (p % 64) // 2
    invf_np = np.zeros((PC, 1), dtype=np.float32)
    for p in range(PC):
        i = (p % HD) // 2
        invf_np[p, 0] = 1.0 / (ROPE_THETA ** (2.0 * i / HD))

    rt_d = nc.inline_tensor(rt_np, "rt_c")
    ones64_np = np.ones((1, HD), dtype=np.float32)
    ones64_d = nc.inline_tensor(ones64_np, "ones64_c")
    mask_d = nc.inline_tensor(mask_np, "mask_c")
    invf_d = nc.inline_tensor(invf_np, "invf_c")

    fp32 = dt.float32
    bf16 = dt.bfloat16
    fp16 = dt.float16

    with tile.TileContext(nc) as tc:
        with (
            tc.tile_pool(name="consts", bufs=1) as consts,
            tc.tile_pool(name="resid", bufs=1) as resid,
            tc.tile_pool(name="xp", bufs=2) as xp,
            tc.tile_pool(name="work", bufs=2) as work,
            tc.tile_pool(name="vst", bufs=2) as vst,
            tc.tile_pool(name="stg", bufs=2) as stg,
            tc.tile_pool(name="csw", bufs=1) as csw,
            tc.tile_pool(name="ptri", bufs=2) as ptri_pool,
            tc.tile_pool(name="bigps", bufs=2, space="PSUM") as bigps,
            tc.tile_pool(name="pvps", bufs=2, space="PSUM") as pvps,
            tc.tile_pool(name="accps", bufs=2, space="PSUM") as accps,
        ):
            # ---- load constants / weights to SBUF ----
            # x chunk 0 DMA first so the PE can start ASAP; weights on
            # separate queues so wq doesn't queue behind everything.
            xt0 = xp.tile([128, KT, TC], bf16, tag="xt")
            nc.scalar.dma_start(out=xt0.rearrange("p a b -> p (a b)"),
                                in_=x_in[0, :, :])

            wq_sb = consts.tile([128, KT, PC], bf16, tag="wq")
            wk_sb = consts.tile([128, KT, PC], bf16, tag="wk")
            wv_sb = consts.tile([128, KT, PC], bf16, tag="wv")
            nc.sync.dma_start(
                out=wq_sb, in_=wq_in.ap().rearrange("(kt p) m -> p kt m", p=128))
            nc.gpsimd.dma_start(
                out=wk_sb, in_=wk_in.ap().rearrange("(kt p) m -> p kt m", p=128))
            nc.scalar.dma_start(
                out=wv_sb, in_=wv_in.ap().rearrange("(kt p) m -> p kt m", p=128))
            wo_sb = consts.tile([128, D], bf16, tag="wo")
            nc.gpsimd.dma_start(out=wo_sb, in_=wo_in[:, :])
            rt_sb = consts.tile([128, 128], bf16, tag="rt")
            nc.sync.dma_start(out=rt_sb, in_=rt_d[:, :])
            mask_sb = consts.tile([128, 128], bf16, tag="mask")
            nc.sync.dma_start(out=mask_sb, in_=mask_d[:, :])
            ones64f_sb = consts.tile([1, HD], fp32, tag="ones64")
            nc.sync.dma_start(out=ones64f_sb, in_=ones64_d[:, :])
            invf_sb = consts.tile([128, 1], fp32, tag="invf")
            nc.sync.dma_start(out=invf_sb, in_=invf_d[:, :])
            bq_sb = consts.tile([128, 1], fp32, tag="bq")
            nc.sync.dma_start(out=bq_sb, in_=bq_in.ap().rearrange("(p o) -> p o", o=1))
            bk_sb = consts.tile([128, 1], fp32, tag="bk")
            nc.sync.dma_start(out=bk_sb, in_=bk_in.ap().rearrange("(p o) -> p o", o=1))
            bv_sb = consts.tile([128, 1], fp32, tag="bv")
            nc.sync.dma_start(out=bv_sb, in_=bv_in.ap().rearrange("(p o) -> p o", o=1))
            halfpi_sb = consts.tile([128, 1], fp32, tag="halfpi")
            nc.vector.memset(halfpi_sb, HALF_PI)

            # ---- residents ----
            # qT/kT rotate through 2 buffers (proj of batch b+1 overlaps
            # scores of batch b); hT stays per-batch (read by o-proj two
            # problems later).
            qkh = {}

            def get_qk(bb, which):
                if (bb, which) not in qkh:
                    t = resid.tile([128, S], bf16, tag=which, bufs=2,
                                   name=f"{which}{bb}")
                    qkh[(bb, which)] = t
                return qkh[(bb, which)]

            hT = []
            for bb in range(B):
                th = resid.tile([128, S], bf16, tag=f"hT{bb}")
                hT.append(th)
            # v natural as repeating [ones(64) | d_h0(64) | d_h1(64)]
            # 192-col blocks (plus one trailing ones block): head0's P@V
            # lhsT is [ones|d0] (den in psum rows 0-63, h in 64-127) and
            # head1's is [d1|ones-of-next-block] (h in 0-63, den 64-127) --
            # both plain contiguous 128-col slices.
            NTB = NTOK // B            # 16 tok tiles per batch
            VW = NTB * 192 + 64
            vN = []
            for bb in range(B):
                tv = resid.tile([128, VW], bf16, tag=f"vN{bb}")
                vN.append(tv)
                nc.vector.memset(
                    bass.AP(tensor=tv.tensor, offset=tv.offset,
                            ap=[tv.ap[0], [192, NTB + 1], [1, 64]]), 1.0)
            # RoPE cos/sin tables [128, S] bf16, built from positions
            cos_sb = consts.tile([128, S], bf16, tag="cosT")
            sin_sb = consts.tile([128, S], bf16, tag="sinT")

            CS_CH = 256

            def cs_piece(ci):
                """Build cos/sin for position chunk ci (range-reduced Sin)."""
                sl = ts(ci, CS_CH)
                posi = csw.tile([128, CS_CH], dt.int32, tag="posi")
                nc.sync.dma_start(
                    out=posi,
                    in_=bass.AP(tensor=pos_in, offset=ci * CS_CH,
                                ap=[[0, 128], [1, CS_CH]]))
                posf = csw.tile([128, CS_CH], fp32, tag="posf")
                nc.vector.tensor_copy(posf, posi)
                ang = csw.tile([128, CS_CH], fp32, tag="ang")
                nc.vector.tensor_scalar_mul(ang, posf, invf_sb)
                # sin: reduce ang to [-pi, pi]
                rnd = csw.tile([128, CS_CH], fp32, tag="rnd")
                red = csw.tile([128, CS_CH], fp32, tag="red")
                nc.vector.tensor_scalar(rnd, ang, INV_2PI, MAGIC,
                                        mybir.AluOpType.mult,
                                        mybir.AluOpType.add)
                nc.vector.tensor_scalar(rnd, rnd, MAGIC, None,
                                        mybir.AluOpType.subtract)
                nc.vector.scalar_tensor_tensor(
                    red, rnd, -TWO_PI, ang,
                    op0=mybir.AluOpType.mult, op1=mybir.AluOpType.add)
                nc.scalar.activation(sin_sb[:, sl], red,
                                     mybir.ActivationFunctionType.Sin)
                # cos(x) = sin(y + pi/2), y = x - 2pi*round((x+pi/2)/2pi)
                nc.vector.tensor_scalar(rnd, ang, INV_2PI, MAGIC + 0.25,
                                        mybir.AluOpType.mult,
                                        mybir.AluOpType.add)
                nc.vector.tensor_scalar(rnd, rnd, MAGIC, None,
                                        mybir.AluOpType.subtract)
                nc.vector.scalar_tensor_tensor(
                    red, rnd, -TWO_PI, ang,
                    op0=mybir.AluOpType.mult, op1=mybir.AluOpType.add)
                nc.scalar.activation(cos_sb[:, sl], red,
                                     mybir.ActivationFunctionType.Sin,
                                     bias=halfpi_sb[:, :])

            # ---- phase 1: QKV projection + RoPE, per token chunk ----
            xt_cache = {0: xt0}

            def get_xt(tci):
                if tci not in xt_cache:
                    xt = xp.tile([128, KT, TC], bf16, tag="xt")
                    nc.scalar.dma_start(out=xt.rearrange("p a b -> p (a b)"),
                                        in_=x_in[tci, :, :])
                    xt_cache[tci] = xt
                return xt_cache[tci]

            def proj_qk_mm(tci, which, st):
                """q/k projection matmuls for one 512-token chunk."""
                xt = get_xt(tci)
                if tci + 1 < NTC:
                    get_xt(tci + 1)
                w_sb, b_sb = ((wq_sb, bq_sb) if which == "q"
                              else (wk_sb, bk_sb))
                pa = accps.tile([128, TC], fp32, tag="acc")
                for kt in range(KT):
                    nc.tensor.matmul(pa, lhsT=w_sb[:, kt, :],
                                     rhs=xt[:, kt, :],
                                     start=(kt == 0), stop=(kt == KT - 1))
                a_sb = work.tile([128, TC], bf16, tag="a_sb")
                nc.vector.tensor_scalar_add(a_sb, pa, b_sb)
                st[which] = a_sb

            def proj_qk_rope(tci, which, st):
                """RoPE for the chunk: rotation matmul + combine."""
                dest = get_qk(tci // CPB, which)
                a_sb = st.pop(which)
                pb = accps.tile([128, TC], fp32, tag="acc")
                nc.tensor.matmul(pb, lhsT=rt_sb, rhs=a_sb,
                                 start=True, stop=True)
                ssl = ds((tci * TC) % S, TC)
                t1 = work.tile([128, TC], bf16, tag="t1")
                nc.vector.tensor_mul(t1, a_sb, cos_sb[:, ssl])
                t2 = work.tile([128, TC], bf16, tag="t2")
                nc.vector.tensor_mul(t2, pb, sin_sb[:, ssl])
                nc.gpsimd.tensor_add(dest[:, ts(tci % CPB, TC)], t1, t2)

            def proj_v(tci):
                """v projection, transposed production (wv stationary),
                then DMA-transposed into natural layout vN on sync."""
                xt = get_xt(tci)
                pv = accps.tile([128, TC], fp32, tag="acc")
                for kt in range(KT):
                    nc.tensor.matmul(pv, lhsT=wv_sb[:, kt, :],
                                     rhs=xt[:, kt, :],
                                     start=(kt == 0), stop=(kt == KT - 1))
                vTst = vst.tile([128, TC], bf16, tag="vTst")
                nc.vector.tensor_scalar_add(vTst, pv, bv_sb)
                for sub in range(TC // 128):
                    tl = (tci % CPB) * (TC // 128) + sub
                    nc.sync.dma_start_transpose(
                        vN[tci // CPB][:, ds(192 * tl + 64, 128)],
                        vTst[:, ts(sub, 128)])

            def proj_pieces(b):
                out = []
                for cc in range(CPB):
                    tci = b * CPB + cc
                    st = {}
                    out.append(lambda t=tci, s=st: proj_qk_mm(t, "q", s))
                    out.append(lambda t=tci, s=st: proj_qk_rope(t, "q", s))
                    out.append(lambda t=tci, s=st: proj_qk_mm(t, "k", s))
                    out.append(lambda t=tci, s=st: proj_qk_rope(t, "k", s))
                    out.append(lambda t=tci: proj_v(t))
                return out

            # ---- phase 2a: scores + exp + mask for one (batch, head) ----
            def score_pieces(i, pt):
                """Closures, each = one <=1024-col psum group: matmul the
                row pieces intersecting the window, exp into pt, mask
                any diagonal blocks fully inside the window."""
                b, h = i // HPC, i % HPC
                hsl = ds(h * HD, HD)
                kTb = get_qk(b, "k")
                qTb = get_qk(b, "q")
                pieces = []
                x0 = 0
                while x0 < PTRI_W:
                    w = min(SGW, PTRI_W - x0)

                    def piece(x0=x0, w=w):
                        sc = bigps.tile([128, SGW], fp32, tag="big")
                        # rows intersecting flat-span window [x0, x0+w)
                        for kt in range(NQT):
                            r0, r1 = OFFS[kt], OFFS[kt] + (S - kt * 128)
                            lo, hi = max(r0, x0), min(r1, x0 + w)
                            if lo >= hi:
                                continue
                            q0 = kt * 128 + (lo - r0)
                            # split at 512-aligned psum columns: a matmul
                            # output must not cross a PSUM bank boundary
                            cuts = [lo]
                            nb = (lo - x0) // 512 * 512 + 512 + x0
                            while nb < hi:
                                cuts.append(nb)
                                nb += 512
                            cuts.append(hi)
                            for aa, bb in zip(cuts, cuts[1:]):
                                nc.tensor.matmul(
                                    sc[:, ds(aa - x0, bb - aa)],
                                    lhsT=kTb[hsl, ds(kt * 128, 128)],
                                    rhs=qTb[hsl, ds(q0 + (aa - lo), bb - aa)],
                                    start=True, stop=True)
                        nc.scalar.activation(
                            pt[:, ds(x0, w)], sc[:, 0:w],
                            mybir.ActivationFunctionType.Exp, scale=SCALE)
                        for kt in range(NQT):
                            if x0 <= OFFS[kt] and OFFS[kt] + 128 <= x0 + w:
                                dsl = ds(OFFS[kt], 128)
                                nc.gpsimd.tensor_mul(
                                    pt[:, dsl], pt[:, dsl], mask_sb)

                    pieces.append(piece)
                    x0 += w
                return pieces

            # ---- phase 2b: P@V (V stationary) + normalize into hT ----
            # Each P@V chunk's psum acc [128,512] (den rows + h rows) is
            # cast once to fp16 staging; den rows gather into [16,128],
            # DMA-transpose to [128,16], DVE reciprocal -> fp16, one DMA
            # scatters to rec_row [1,S] (q-order), and a K=1 ones-matmul
            # broadcasts 1/den to [64,512] psum for the normalize mult.
            def pv_pieces(i, pt):
                b, h = i // HPC, i % HPC
                st = {}
                pieces = []

                def mm_piece(c):
                    q0 = c * QC
                    acc = pvps.tile([128, QC], fp32, tag="pv")
                    kts = [kt for kt in range(NQT) if kt * 128 < q0 + QC]
                    for j, kt in enumerate(kts):
                        lo = max(kt * 128, q0)
                        w = q0 + QC - lo
                        lt = vN[b][:, ds(192 * kt + 128 * h, 128)]
                        nc.tensor.matmul(
                            acc[:, ds(lo - q0, w)], lhsT=lt,
                            rhs=pt[:, ds(OFFS[kt] + lo - kt * 128, w)],
                            start=(j == 0), stop=(j == len(kts) - 1))
                    if c == 0:
                        d16 = stg.tile([16, 128], fp16, tag="den16")
                        st["den16"] = d16
                    sacc = stg.tile([128, QC], fp16, tag="sacc", bufs=5)
                    nc.vector.tensor_copy(sacc, acc)
                    st[("hu", c)] = sacc[ds(64 - 64 * h, HD), :]
                    nc.sync.dma_start(
                        out=st["den16"][ds(4 * c, 4), :],
                        in_=sacc[ds(64 * h, 1), :])

                def rec_piece():
                    # den16 rows are already q-ordered: den16[R, m] =
                    # den(q = R*128 + m). Reciprocal in this layout (16
                    # lanes), then one partition-major DMA linearizes to
                    # rec_row[0, q].
                    den32 = stg.tile([16, 128], fp32, tag="den32")
                    nc.vector.tensor_copy(den32, st["den16"])
                    rec32 = stg.tile([16, 128], fp32, tag="rec32")
                    nc.vector.reciprocal_approx_fast(rec32, den32)
                    rr = stg.tile([1, S], fp32, tag="rr", bufs=2)
                    nc.sync.dma_start(
                        out=bass.AP(tensor=rr.tensor, offset=rr.offset,
                                    ap=[[rr.ap[0][0], 1], [1, S]]),
                        in_=rec32[:, :])
                    for c in range(NPV):
                        recb = pvps.tile([HD, QC], fp32, tag="pv")
                        nc.tensor.matmul(
                            recb, lhsT=ones64f_sb[0:1, :],
                            rhs=rr[0:1, ds(c * QC, QC)],
                            start=True, stop=True)
                        nc.vector.tensor_mul(
                            hT[b][ds(h * HD, HD), ds(c * QC, QC)],
                            st[("hu", c)], recb)
                    if KDEBUG and i == 0:
                        nc.sync.dma_start(out=dbg_rr[:, :], in_=rr)

                for c in range(NPV):
                    pieces.append(lambda c=c: mm_piece(c))
                pieces.append(rec_piece)
                return pieces

            # ---- phase 3: output projection (partial, transposed) ----
            def oproj_piece(b, ft):
                for cc in range(S // 512):
                    po = accps.tile([128, 512], fp32, tag="acc")
                    nc.tensor.matmul(
                        po, lhsT=wo_sb[:, ts(ft, 128)],
                        rhs=hT[b][:, ds(cc * 512, 512)],
                        start=True, stop=True)
                    ostage = work.tile([128, 512], bf16, tag="ostage")
                    nc.vector.tensor_copy(ostage, po)
                    nc.gpsimd.dma_start(
                        out=out_d[ft, b * (S // 512) + cc, :, :], in_=ostage)

            def oproj_pieces(b, fts):
                return [lambda f=ft, bb=b: oproj_piece(bb, f) for ft in fts]

            # ---- emission schedule ----
            def interleave(main, others, ratio=None):
                """Emit main pieces with `others` spliced evenly."""
                if ratio is None:
                    ratio = max(1, len(main) // max(1, len(others)))
                oi = 0
                for n, m in enumerate(main):
                    m()
                    if n % ratio == ratio - 1 and oi < len(others):
                        others[oi]()
                        oi += 1
                for o in others[oi:]:
                    o()

            # startup: batch-0 projection with cos/sin builds interleaved
            p0 = proj_pieces(0)
            for n, p in enumerate(p0):
                if n % 5 == 0 and n // 5 < 4:
                    cs_piece(2 * (n // 5))
                    cs_piece(2 * (n // 5) + 1)
                p()

            nprob = B * HPC
            pts = {}
            pvq = []      # pending P@V pieces (from previous problem)
            for i in range(nprob + 1):
                fillers = list(pvq)
                pvq = []
                # projection for batch i//2+1 split over iterations 2b-2,
                # 2b-1; O-proj for batch (i-3)//2 split over 2b+3, 2b+4
                bb = i // 2 + 1
                half = i % 2
                if bb < B:
                    fillers += proj_pieces(bb)[10 * half:10 * half + 10]
                if i >= 3:
                    ob = (i - 3) // 2
                    ohalf = (i - 3) % 2
                    fillers += oproj_pieces(ob, range(4 * ohalf, 4 * ohalf + 4))
                if i == nprob:   # epilogue: last batch's O-proj
                    fillers += oproj_pieces(B - 1, range(8))
                if i < nprob:
                    pt = ptri_pool.tile([128, PTRI_W], bf16, tag="pt")
                    pts[i] = pt
                    interleave(score_pieces(i, pt), fillers)
                    pvq = pv_pieces(i, pt)
                else:
                    interleave(fillers, [])
            # pv of the last problem ran inside the epilogue fillers
            if KDEBUG:
                for bb in range(B):
                    nc.sync.dma_start(out=dbg_q[:, ts(bb, S)],
                                      in_=get_qk(bb, "q"))
                    nc.sync.dma_start(out=dbg_k[:, ts(bb, S)],
                                      in_=get_qk(bb, "k"))
                    nc.sync.dma_start(out=dbg_h[:, ts(bb, S)], in_=hT[bb])

    nc.compile()
    return nc


_NC_CACHE = None


def _get_nc():
    global _NC_CACHE
    if _NC_CACHE is None:
        _NC_CACHE = _build_nc()
    return _NC_CACHE


def build_in_maps(x, positions, Wqkv, bqkv, Wo, bo):
    xT = x.reshape(T, D).T.astype(BF16)            # [D, T]
    # chunk-block layout [tci, p, kt*512]: contiguous 4KB runs per partition
    xblk = np.ascontiguousarray(
        xT.reshape(KT, 128, NTC, TC).transpose(2, 1, 0, 3).reshape(NTC, 128, KT * TC))
    pos = np.ascontiguousarray(positions[0]).astype(np.int32)
    in_maps = []
    for c in range(NCORES):
        r0 = c * PC
        wq = np.ascontiguousarray(Wqkv[r0:r0 + PC, :].T).astype(BF16)
        wk = np.ascontiguousarray(Wqkv[D + r0:D + r0 + PC, :].T).astype(BF16)
        wv = np.ascontiguousarray(Wqkv[2 * D + r0:2 * D + r0 + PC, :].T).astype(BF16)
        wo = np.ascontiguousarray(Wo[:, r0:r0 + PC].T).astype(BF16)
        in_maps.append({
            "x": xblk, "pos": pos,
            "wq": wq, "wk": wk, "wv": wv, "wo": wo,
            "bq": bqkv[r0:r0 + PC].astype(np.float32),
            "bk": bqkv[D + r0:D + r0 + PC].astype(np.float32),
            "bv": bqkv[2 * D + r0:2 * D + r0 + PC].astype(np.float32),
        })
    return in_maps


def assemble_out(res, bo):
    acc = res.results[0]["out"].astype(np.float32)
    for c in range(1, NCORES):
        acc += res.results[c]["out"].astype(np.float32)
    # [KT, T//512, 128, 512] -> [D, T]
    full = acc.transpose(0, 2, 1, 3).reshape(D, T)
    out = full + bo[:, None].astype(np.float32)
    return np.ascontiguousarray(out.T).reshape(B, S, D)


def kernel(x, positions, Wqkv, bqkv, Wo, bo):
    x = np.asarray(x)
    positions = np.asarray(positions)
    Wqkv = np.asarray(Wqkv)
    bqkv = np.asarray(bqkv)
    Wo = np.asarray(Wo)
    bo = np.asarray(bo)
    nc = _get_nc()
    in_maps = build_in_maps(x, positions, Wqkv, bqkv, Wo, bo)
    res = run_bass_kernel_spmd(nc, in_maps, core_ids=list(range(NCORES)))
    return assemble_out(res, bo)
